# revision 48
# baseline (speedup 1.0000x reference)
"""Distributed Trainium2 kernel for causal GQA attention with RoPE.

Model: B=2, S=2048, DM=2048, H=16 q-heads, HK=4 kv-heads, D=128.
Sharding over 8 NeuronCores: core c = (batch b=c//4, kv-head kh=c%4).
Each core computes its 4 q-heads / 1 kv-head of one batch end-to-end,
AllGathers attention outputs within its 4-core batch group, and applies
a column slice of Wo, producing out[b][:, kh*512:(kh+1)*512].
Attention for quarter q runs right after projection chunk q so the
gathers fire early; quarter 3 is exchanged per-head to shrink the tail.
"""
import contextlib
import ctypes
import os
import sys
import types

for _p in ("/opt/trn_rl_repo", "/root/.axon_site/_ro/trn_rl_repo"):
    if os.path.isdir(_p) and _p not in sys.path:
        sys.path.insert(0, _p)

import numpy as np
import ml_dtypes

import concourse.bass as bass
import concourse.mybir as mybir
import concourse.tile as tile
from concourse import bacc
from concourse.bass import ts, ds
from concourse.bass_utils import run_bass_kernel_spmd
from concourse.masks import make_identity

BF16 = ml_dtypes.bfloat16
F32 = mybir.dt.float32
BF = mybir.dt.bfloat16

B, S, DM = 2, 2048, 2048
H, HK, D = 16, 4, 128
G = H // HK          # q heads per kv head (= heads per core)
THETA = 10000.0
N_CORES = 8
KT = DM // 128       # 16 K-tiles of the model dim
TOKB = S // 128      # 16 token blocks
TCH = S // 512       # 4 token chunks of 512
HD_CORE = G * D      # 512 output dims of q per core
NEG = -1.0e30

LAST_EXEC_TIME_NS = None
LAST_RESULTS = None


# ---------------------------------------------------------------- tracing
def _install_ntff_hook():
    """Make run_bass_kernel_spmd(trace=True) work in this container."""
    try:
        from antenv.axon_hooks import get_axon_ntff_profile_hook  # noqa: F401
        return True
    except ImportError:
        pass
    so_path = "/opt/axon/libaxon_pjrt.so"
    if not os.path.exists(so_path):
        return False
    lib = ctypes.CDLL(so_path)
    if not hasattr(lib, "axon_start_nrt_profile"):
        return False
    lib.axon_start_nrt_profile.argtypes = [ctypes.POINTER(ctypes.c_int64), ctypes.c_size_t]
    lib.axon_start_nrt_profile.restype = ctypes.c_int64
    lib.axon_stop_nrt_profile.argtypes = [ctypes.c_char_p]
    lib.axon_stop_nrt_profile.restype = ctypes.c_int64

    @contextlib.contextmanager
    def _hook(output_dir, device_ids):
        import jax
        jax.devices()
        if device_ids:
            ids = (ctypes.c_int64 * len(device_ids))(*device_ids)
            rc = lib.axon_start_nrt_profile(ids, len(device_ids))
        else:
            rc = lib.axon_start_nrt_profile(None, 0)
        if rc != 0:
            raise RuntimeError(f"axon_start_nrt_profile rc={rc}")
        try:
            yield
        finally:
            n = lib.axon_stop_nrt_profile(str(output_dir).encode())
            print(f"profile: {n} file(s) in {output_dir}", file=sys.stderr)

    mod = types.ModuleType("antenv.axon_hooks")
    holder = {"h": _hook}
    mod.set_axon_ntff_profile_hook = lambda h: holder.__setitem__("h", h)
    mod.get_axon_ntff_profile_hook = lambda: holder.get("h")
    sys.modules["antenv.axon_hooks"] = mod
    import antenv
    antenv.axon_hooks = mod
    import concourse.bass_utils as bu
    bu.upload_artifacts = lambda tmpdir: str(tmpdir)
    return True



# ---------------------------------------------------------------- graph
def build_nc():
    nc = bacc.Bacc("TRN2", target_bir_lowering=False, debug=False,
                   num_devices=N_CORES)

    rotm = nc.dram_tensor("rotm", [D, D], BF, kind="ExternalInput").ap()
    xt = nc.dram_tensor("xt", [DM, S], BF, kind="ExternalInput").ap()
    wq = nc.dram_tensor("wq", [DM, HD_CORE], BF, kind="ExternalInput").ap()
    wk = nc.dram_tensor("wk", [DM, D], BF, kind="ExternalInput").ap()
    wv = nc.dram_tensor("wv", [DM, D], BF, kind="ExternalInput").ap()
    wo = nc.dram_tensor("wo", [DM, HD_CORE], BF, kind="ExternalInput").ap()
    cosq = nc.dram_tensor("cosq", [D, S], F32, kind="ExternalInput").ap()
    sinq = nc.dram_tensor("sinq", [D, S], F32, kind="ExternalInput").ap()
    cosk = nc.dram_tensor("cosk", [D, S], F32, kind="ExternalInput").ap()
    sink = nc.dram_tensor("sink", [D, S], F32, kind="ExternalInput").ap()
    out = nc.dram_tensor("out", [S, HD_CORE], F32, kind="ExternalOutput").ap()

    groups = [[0, 1, 2, 3], [4, 5, 6, 7]]

    with tile.TileContext(nc) as tc:
        with tc.tile_pool(name="const", bufs=1) as cpool, \
             tc.tile_pool(name="tblp", bufs=2) as tpool, \
             tc.tile_pool(name="wts", bufs=1) as wpool, \
             tc.tile_pool(name="acts", bufs=1) as apool, \
             tc.tile_pool(name="xin", bufs=36) as xpool, \
             tc.tile_pool(name="work", bufs=3) as work, \
                          tc.tile_pool(name="etwork", bufs=9) as etwork, \
             tc.tile_pool(name="ogp", bufs=32) as ogpool, \
             tc.tile_pool(name="stats", bufs=2) as stats, \
             tc.tile_pool(name="bcp", bufs=2) as bcpool, \
             tc.tile_pool(name="psmm", bufs=4, space="PSUM") as ps_mm, \
             tc.tile_pool(name="pspv", bufs=2, space="PSUM") as ps_pv, \
             tc.tile_pool(name="psden", bufs=1, space="PSUM") as ps_den, \
             tc.tile_pool(name="pstr", bufs=1, space="PSUM") as ps_tr, \
             tc.tile_pool(name="dram", bufs=1, space="DRAM") as dpool:

            # warm up the collective path FIRST: the gpsimd engine is the
            # collective-trigger path, so nothing may sit ahead of this in
            # its queue (a late trigger delays the group-wide init barrier
            # and cascades into every later gather)
            warm_in = dpool.tile([128, 4], F32, tag="warm_in", name="warm_in")
            warm_out = dpool.tile([4, 128, 4], F32, tag="warm_out",
                                  name="warm_out")
            nc.gpsimd.dma_start(out=warm_in[:], in_=cosq[0:128, 0:4])
            nc.gpsimd.collective_compute(
                "AllGather", mybir.AluOpType.bypass,
                replica_groups=groups,
                ins=[warm_in.opt()], outs=[warm_out.opt()])

            # HAM warm-up: ~100 dummy matmuls on a zero tile keep the PE
            # busy through the initial DMA ramp so the clock gate opens
            # (4/8 -> 8/8) before the first real projection matmuls
            hamw = cpool.tile([128, 128], BF, tag="hamw", name="hamw")
            nc.vector.memset(hamw[:], 0.0)
            for _ in range(60):
                psd = ps_mm.tile([128, 512], F32, tag="mm", name="mm")
                nc.tensor.matmul(psd[:, 0:128], hamw[:], hamw[:],
                                 start=True, stop=True)

            # ---------------- constants
            ident = cpool.tile([128, 128], BF, tag="ident", name="ident")
            make_identity(nc, ident[:])
            # transposed causal mask: keep [k_row p, q_col j] iff j >= p
            cmaskT = cpool.tile([128, 128], F32, tag="cmaskT", name="cmaskT")
            nc.gpsimd.memset(cmaskT[:], 0.0)
            nc.gpsimd.affine_select(
                out=cmaskT[:], in_=cmaskT[:],
                compare_op=mybir.AluOpType.is_ge, fill=NEG,
                base=0, pattern=[[1, 128]], channel_multiplier=-1)
            ones_sb = cpool.tile([128, 1], BF, tag="ones", name="ones")
            nc.gpsimd.memset(ones_sb[:], 1.0)
            # warm the ACT exp table during the DMA ramp (the first real exp
            # otherwise pays the ~2.7us table load inside attn(0)'s chain)
            warm_act = cpool.tile([1, 4], F32, tag="warm_act", name="warm_act")
            nc.scalar.activation(out=warm_act[0:1, 0:1], in_=ones_sb[0:1, 0:1],
                                 func=mybir.ActivationFunctionType.Exp)
            # signed rotate-half permutation (lhsT): sh = rotm.T @ raw
            rot_sb = cpool.tile([D, D], BF, tag="rotm", name="rotm")

            # ---------------- weights + first x chunk (load order = use order)
            wq_sb = [wpool.tile([128, HD_CORE], BF, tag=f"wq{kt}",
                                name=f"wq{kt}") for kt in range(KT)]
            wk_sb = [wpool.tile([128, D], BF, tag=f"wk{kt}",
                                name=f"wk{kt}") for kt in range(KT)]
            wv_sb = [wpool.tile([128, D], BF, tag=f"wv{kt}",
                                name=f"wv{kt}") for kt in range(KT)]
            wo_sb = [wpool.tile([128, HD_CORE], BF, tag=f"wo{kt}",
                                name=f"wo{kt}") for kt in range(KT)]

            def load_xc(c):
                ts_ = [xpool.tile([128, 512], BF, tag="xc", name="xc")
                       for _ in range(KT)]
                for kt in range(KT):
                    # interleave weight loads on first chunk so matmul kt can
                    # start as soon as its slices land
                    nc.sync.dma_start(
                        out=ts_[kt][:],
                        in_=xt[ds(128 * kt, 128), ds(512 * c, 512)])
                    if c == 0:
                        nc.sync.dma_start(out=wk_sb[kt][:],
                                          in_=wk[ds(128 * kt, 128), :])
                return ts_

            def load_tbl(c):
                """rope-table slices for chunk c (sync queue, use order)."""
                tb = {}
                for name, src in (("cosk", cosk), ("sink", sink),
                                  ("cosq", cosq), ("sinq", sinq)):
                    t = tpool.tile([D, 512], F32, tag=f"tb_{name}")
                    nc.sync.dma_start(out=t[:], in_=src[:, ds(512 * c, 512)])
                    tb[name] = t
                return tb

            xc_state = [load_xc(0)]
            # rot matrix first (the rope perm-matmul sits in the PE FIFO
            # right after K-proj), then wq so Q-proj head 0 can start as
            # soon as K-proj drains; tables and wv follow
            nc.sync.dma_start(out=rot_sb[:], in_=rotm[:])
            for kt in range(KT):
                nc.sync.dma_start(out=wq_sb[kt][:],
                                  in_=wq[ds(128 * kt, 128), :])
            tbl_state = [load_tbl(0)]
            for kt in range(KT):
                nc.sync.dma_start(out=wv_sb[kt][:],
                                  in_=wv[ds(128 * kt, 128), :])
            # chunk 1 streams in behind the weights so projection chunk
            # boundaries never wait on x arrival (two chunks in flight)
            xc_state.append(load_xc(1))
            tbl_state.append(load_tbl(1))

            # ---------------- persistent activations
            qt_sb = [apool.tile([D, S], BF, tag=f"qt{h}", name=f"qt{h}")
                     for h in range(G)]
            kt_sb = apool.tile([D, S], BF, tag="kt", name="kt")
            vtok_sb = apool.tile([128, TOKB, D], BF, tag="vtok", name="vtok")

            # ---------------- projections + RoPE + v transpose
            def rope_store(raw_ps, dst_slice, cos_t, sin_t):
                # rotate-half via a PE permutation matmul (no DMA in chain);
                # t2/add ride the idle gpsimd ALU to decongest VectorE
                raw = work.tile([128, 512], BF, tag="qraw", name="qraw")
                nc.scalar.copy(raw[:], raw_ps[:])
                sh_ps = ps_pv.tile([128, 512], F32, tag="pv", name="ropesh")
                nc.tensor.matmul(sh_ps[:], rot_sb[:], raw[:],
                                 start=True, stop=True)
                t1 = work.tile([128, 512], F32, tag="t1", name="t1")
                nc.vector.tensor_mul(t1[:], sh_ps[:], sin_t[:])
                t2 = work.tile([128, 512], F32, tag="t2", name="t2")
                nc.vector.tensor_mul(t2[:], raw[:], cos_t[:])
                nc.vector.tensor_add(dst_slice, t1[:], t2[:])

            def emit_proj(c):
                xc = xc_state.pop(0)
                tbl = tbl_state.pop(0)
                # k
                ps = ps_mm.tile([128, 512], F32, tag="mm", name="mm")
                for kt in range(KT):
                    nc.tensor.matmul(ps[:], wk_sb[kt][:], xc[kt][:],
                                     start=(kt == 0), stop=(kt == KT - 1))
                rope_store(ps, kt_sb[:, ds(512 * c, 512)],
                           tbl["cosk"], tbl["sink"])
                # q heads
                for h in range(G):
                    ps = ps_mm.tile([128, 512], F32, tag="mm", name="mm")
                    for kt in range(KT):
                        nc.tensor.matmul(ps[:], wq_sb[kt][:, ts(h, 128)],
                                         xc[kt][:],
                                         start=(kt == 0), stop=(kt == KT - 1))
                    rope_store(ps, qt_sb[h][:, ds(512 * c, 512)],
                               tbl["cosq"], tbl["sinq"])
                # v last (no rope; transpose to token-major)
                ps = ps_mm.tile([128, 512], F32, tag="mm", name="mm")
                for kt in range(KT):
                    nc.tensor.matmul(ps[:], wv_sb[kt][:], xc[kt][:],
                                     start=(kt == 0), stop=(kt == KT - 1))
                vst = work.tile([128, 512], BF, tag="vst", name="vst")
                nc.scalar.copy(vst[:], ps[:])
                trp = ps_tr.tile([128, 512], BF, tag="tr", name="trv")
                for j in range(4):
                    nc.tensor.transpose(trp[:, ts(j, 128)], vst[:, ts(j, 128)],
                                        ident[:])
                nc.vector.tensor_copy(out=vtok_sb[:, ds(4 * c, 4), :], in_=trp[:])
                if c + 2 < TCH:
                    xc_state.append(load_xc(c + 2))
                    tbl_state.append(load_tbl(c + 2))

            # ---------------- attention, scores computed pre-transposed
            # quarters 0-2 gather all 4 local heads at quarter end; quarter 3
            # gathers per head so the final Wo only waits on 128KB.
            cin_q = [dpool.tile([D, G, 512], BF, tag=f"cinq{t}", name=f"cinq{t}")
                     for t in range(3)]
            cout_q = [dpool.tile([4, D, G, 512], BF, tag=f"coutq{t}",
                                 name=f"coutq{t}") for t in range(3)]
            cin_q3 = [dpool.tile([D, 2, 512], BF, tag=f"cinq3p{p}",
                                 name=f"cinq3p{p}") for p in range(2)]
            cout_q3 = [dpool.tile([4, D, 2, 512], BF, tag=f"coutq3p{p}",
                                  name=f"coutq3p{p}") for p in range(2)]

            def wo_quarter(t, part=None):
                """Load og tiles for quarter t (part: q3 head-pair 0/1)."""
                if t < 3:
                    srcs = [(r * G + h, cout_q[t][r, :, h, :])
                            for r in range(4) for h in range(G)]
                else:
                    srcs = [(r * G + 2 * part + h, cout_q3[part][r, :, h, :])
                            for h in range(2) for r in range(4)]
                srcs.sort()
                ogs = []
                for kt, ap in srcs:
                    og = ogpool.tile([128, 512], BF, tag="og", name="og")
                    # gpsimd queue: a pending gather-wait here must not block
                    # the sync queue's x/weight streams (head-of-line)
                    nc.gpsimd.dma_start(out=og[:], in_=ap)
                    ogs.append((kt, og))
                return ogs

            def wo_mm(t, ogs, ogs2=None):
                """ogs2: late-arriving og tiles; all chains' ogs-matmuls are
                emitted before any ogs2 matmul so the PE stalls at most once
                on the final gather."""
                n1 = len(ogs)
                pws = []
                for tb in range(4):
                    pw = ps_mm.tile([128, 512], F32, tag="mm", name="mm")
                    pws.append(pw)
                    for idx, (kt, og) in enumerate(ogs):
                        nc.tensor.matmul(pw[:], og[:, ts(tb, 128)],
                                         wo_sb[kt][:],
                                         start=(idx == 0), stop=(idx == 15))
                for tb in range(4):
                    pw = pws[tb]
                    if ogs2 is not None:
                        for j, (kt, og) in enumerate(ogs2):
                            nc.tensor.matmul(pw[:], og[:, ts(tb, 128)],
                                             wo_sb[kt][:],
                                             start=False,
                                             stop=(n1 + j == 15))
                    ost = work.tile([128, 512], F32, tag="ost", name="ost")
                    nc.scalar.copy(ost[:], pw[:])
                    nc.sync.dma_start(
                        out=out[ds(512 * t + 128 * tb, 128), :], in_=ost[:])

            def emit_st(h, qc, kb):
                """score block, transposed: [k 128, q<=512] -> exp -> et"""
                band = kb - 4 * qc
                et = etwork.tile([128, 512], BF, tag="et", name="et")
                sps = ps_mm.tile([128, 512], F32, tag="mm", name="mm")
                if band >= 0:
                    off = 128 * band
                    w = 512 - off
                    nc.tensor.matmul(sps[:, :w], kt_sb[:, ts(kb, 128)],
                                     qt_sb[h][:, ds(512 * qc + off, w)],
                                     start=True, stop=True)
                    nc.vector.tensor_add(sps[:, :128], sps[:, :128], cmaskT[:])
                    if off:
                        nc.vector.memset(et[:, :off], 0.0)
                    nc.scalar.activation(
                        out=et[:, ds(off, w)], in_=sps[:, :w],
                        func=mybir.ActivationFunctionType.Exp)
                    return et, off
                nc.tensor.matmul(sps[:], kt_sb[:, ts(kb, 128)],
                                 qt_sb[h][:, ds(512 * qc, 512)],
                                 start=True, stop=True)
                nc.scalar.activation(
                    out=et[:], in_=sps[:],
                    func=mybir.ActivationFunctionType.Exp)
                return et, 0

            wo_pend = {}
            # wo og-loads / matmuls for quarter t ride inside later quarters'
            # head loops, leaving ~100us of slack after each gather's trigger
            # so collective-start skew never stalls the PE; quarters 1-3 run
            # after attn(3) as ~45us of PE fill overlapping the final gathers
            loads_at = {(2, 3): 0, (3, 0): 1}
            mms_at = {}

            def emit_attn(qc):
                for h in range(G):
                    if (qc, h) in loads_at:
                        t = loads_at[(qc, h)]
                        wo_pend[t] = wo_quarter(t)
                    if (qc, h) in mms_at:
                        t = mms_at[(qc, h)]
                        wo_mm(t, wo_pend.pop(t))
                    nkb = 4 * qc + 4
                    oT_ps = ps_pv.tile([128, 512], F32, tag="pv", name="pv")
                    den_ps = ps_den.tile([1, 512], F32, tag="den", name="den")
                    pend = [emit_st(h, qc, k) for k in range(min(4, nkb))]
                    ngrp = (nkb + 3) // 4
                    esum = None
                    gready = []
                    emitted = 0
                    for kb in range(nkb):
                        et, off = pend.pop(0)
                        if kb + 4 < nkb:
                            pend.append(emit_st(h, qc, kb + 4))
                        nc.tensor.matmul(oT_ps[:, ds(off, 512 - off)],
                                         vtok_sb[:, kb, :],
                                         et[:, ds(off, 512 - off)],
                                         start=(kb == 0), stop=(kb == nkb - 1))
                        # denominator: sum groups of 4 et tiles on DVE; each
                        # group's ones-matmul is emitted one group LATE so
                        # the PE FIFO never waits on the DVE chain
                        gi, gj = divmod(kb, 4)
                        last_in_grp = (gj == 3 or kb == nkb - 1)
                        if gj == 0:
                            esum = et
                        else:
                            nsum = etwork.tile([128, 512], BF, tag="esum",
                                               name="esum", bufs=6)
                            nc.vector.tensor_add(nsum[:], esum[:], et[:])
                            esum = nsum
                        if last_in_grp:
                            gready.append(esum)
                            if len(gready) > 1:
                                nc.tensor.matmul(den_ps[:], ones_sb[:, 0:1],
                                                 gready.pop(0)[:],
                                                 start=(emitted == 0),
                                                 stop=False)
                                emitted += 1
                    for es in gready:
                        nc.tensor.matmul(den_ps[:], ones_sb[:, 0:1], es[:],
                                         start=(emitted == 0),
                                         stop=(emitted == ngrp - 1))
                        emitted += 1
                    den_sb = stats.tile([1, 512], F32, tag="den_sb",
                                        name="den_sb")
                    nc.scalar.copy(den_sb[:], den_ps[:])
                    rec = stats.tile([1, 512], F32, tag="recq", name="recq")
                    nc.vector.reciprocal(rec[:], den_sb[:])
                    bcast = bcpool.tile([128, 512], F32, tag="bcast",
                                        name="bcast")
                    nc.gpsimd.partition_broadcast(bcast[:], rec[:])
                    otst = work.tile([128, 512], BF, tag="otst", name="otst")
                    nc.vector.tensor_mul(otst[:], oT_ps[:], bcast[:])
                    if qc < 3:
                        nc.sync.dma_start(out=cin_q[qc][:, h, :], in_=otst[:])
                    else:
                        p, hh = divmod(h, 2)
                        nc.sync.dma_start(out=cin_q3[p][:, hh, :], in_=otst[:])
                        if hh == 1:
                            nc.gpsimd.collective_compute(
                                "AllGather", mybir.AluOpType.bypass,
                                replica_groups=groups,
                                ins=[cin_q3[p].opt()],
                                outs=[cout_q3[p].opt()])
                if qc < 3:
                    nc.gpsimd.collective_compute(
                        "AllGather", mybir.AluOpType.bypass,
                        replica_groups=groups,
                        ins=[cin_q[qc].opt()], outs=[cout_q[qc].opt()])

            emit_proj(0)
            emit_attn(0)
            emit_proj(1)
            # wo weights: needed from the first Wo quarter (inside attn(1))
            for kt in range(KT):
                nc.sync.dma_start(out=wo_sb[kt][:],
                                  in_=wo[ds(128 * kt, 128), :])
            emit_attn(1)
            emit_proj(2)
            emit_attn(2)
            emit_proj(3)
            emit_attn(3)
            # tail: ALL wo work runs after attn(3) so the final head-pair
            # gathers trigger as early as possible and complete under ~60us
            # of Wo matmul fill.  og-load order keeps the gpsimd DMA queue
            # from blocking ahead of the pair triggers.
            wo_mm(0, wo_pend.pop(0))
            wo_mm(1, wo_pend.pop(1))
            wo_mm(2, wo_quarter(2))
            wo_mm(3, wo_quarter(3, part=0), wo_quarter(3, part=1))

    nc.finalize()
    return nc


_NC_CACHE = {}


def _get_nc():
    if "nc" not in _NC_CACHE:
        _NC_CACHE["nc"] = build_nc()
    return _NC_CACHE["nc"]


def _rope_tables():
    inv = 1.0 / (THETA ** (np.arange(0, D, 2, dtype=np.float64) / D))  # [64]
    pos = np.arange(S, dtype=np.float64)
    fr = pos[:, None] * inv[None, :]                 # [S, 64]
    emb = np.concatenate([fr, fr], axis=1)           # [S, D]
    cos = np.cos(emb).T.astype(np.float32)           # [D, S]
    sin = np.sin(emb).T.astype(np.float32)           # sign lives in rotm
    scale = np.float32(D ** -0.5)
    return (cos * scale, sin * scale,                # q tables (pre-scaled)
            cos.copy(), sin.copy())                  # k tables


def _rot_matrix():
    """lhsT for sh = rotate_half(raw): sh[i] = -raw[i+64] (i<64),
    raw[i-64] (i>=64);  rotm[p, i] so that sh = rotm.T @ raw."""
    m = np.zeros((D, D), dtype=np.float32)
    half = D // 2
    for i in range(half):
        m[i + half, i] = -1.0
        m[i, i + half] = 1.0
    return m.astype(BF16)


def kernel(x, Wq, Wk, Wv, Wo):
    global LAST_EXEC_TIME_NS, LAST_RESULTS
    nc = _get_nc()
    cq, sq, ck, sk = _rope_tables()
    rm = _rot_matrix()
    in_maps = []
    for c in range(N_CORES):
        b, kh = c // 4, c % 4
        in_maps.append({
            "rotm": rm,
            "xt": np.ascontiguousarray(x[b].T).astype(BF16),
            "wq": np.ascontiguousarray(Wq[:, kh * HD_CORE:(kh + 1) * HD_CORE]).astype(BF16),
            "wk": np.ascontiguousarray(Wk[:, kh * D:(kh + 1) * D]).astype(BF16),
            "wv": np.ascontiguousarray(Wv[:, kh * D:(kh + 1) * D]).astype(BF16),
            "wo": np.ascontiguousarray(Wo[:, kh * HD_CORE:(kh + 1) * HD_CORE]).astype(BF16),
            "cosq": cq, "sinq": sq, "cosk": ck, "sink": sk,
        })
    trace = os.environ.get("KERNEL_TRACE", "0") == "1" and _install_ntff_hook()
    res = run_bass_kernel_spmd(nc, in_maps, core_ids=list(range(N_CORES)),
                               trace=trace)
    LAST_EXEC_TIME_NS = res.exec_time_ns
    LAST_RESULTS = res
    out = np.empty((B, S, DM), dtype=np.float32)
    for c in range(N_CORES):
        b, kh = c // 4, c % 4
        out[b, :, kh * HD_CORE:(kh + 1) * HD_CORE] = res.results[c]["out"]
    return out


# revision 51
# speedup vs baseline: 1.0965x; 1.0965x over previous
"""Distributed Trainium2 kernel for causal GQA attention with RoPE.

Model: B=2, S=2048, DM=2048, H=16 q-heads, HK=4 kv-heads, D=128.
Sharding over 8 NeuronCores: core c = (batch b=c//4, kv-head kh=c%4).
Each core computes its 4 q-heads / 1 kv-head of one batch end-to-end,
AllGathers attention outputs within its 4-core batch group, and applies
a column slice of Wo, producing out[b][:, kh*512:(kh+1)*512].
Attention for quarter q runs right after projection chunk q so the
gathers fire early; quarter 3 is exchanged per-head to shrink the tail.
"""
import contextlib
import ctypes
import os
import sys
import types

for _p in ("/opt/trn_rl_repo", "/root/.axon_site/_ro/trn_rl_repo"):
    if os.path.isdir(_p) and _p not in sys.path:
        sys.path.insert(0, _p)

import numpy as np
import ml_dtypes

import concourse.bass as bass
import concourse.mybir as mybir
import concourse.tile as tile
from concourse import bacc
from concourse.bass import ts, ds
from concourse.bass_utils import run_bass_kernel_spmd
from concourse.masks import make_identity

BF16 = ml_dtypes.bfloat16
F32 = mybir.dt.float32
BF = mybir.dt.bfloat16

B, S, DM = 2, 2048, 2048
H, HK, D = 16, 4, 128
G = H // HK          # q heads per kv head (= heads per core)
THETA = 10000.0
N_CORES = 8
KT = DM // 128       # 16 K-tiles of the model dim
TOKB = S // 128      # 16 token blocks
TCH = S // 512       # 4 token chunks of 512
HD_CORE = G * D      # 512 output dims of q per core
NEG = -1.0e30

LAST_EXEC_TIME_NS = None
LAST_RESULTS = None


# ---------------------------------------------------------------- tracing
def _install_ntff_hook():
    """Make run_bass_kernel_spmd(trace=True) work in this container."""
    try:
        from antenv.axon_hooks import get_axon_ntff_profile_hook  # noqa: F401
        return True
    except ImportError:
        pass
    so_path = "/opt/axon/libaxon_pjrt.so"
    if not os.path.exists(so_path):
        return False
    lib = ctypes.CDLL(so_path)
    if not hasattr(lib, "axon_start_nrt_profile"):
        return False
    lib.axon_start_nrt_profile.argtypes = [ctypes.POINTER(ctypes.c_int64), ctypes.c_size_t]
    lib.axon_start_nrt_profile.restype = ctypes.c_int64
    lib.axon_stop_nrt_profile.argtypes = [ctypes.c_char_p]
    lib.axon_stop_nrt_profile.restype = ctypes.c_int64

    @contextlib.contextmanager
    def _hook(output_dir, device_ids):
        import jax
        jax.devices()
        if device_ids:
            ids = (ctypes.c_int64 * len(device_ids))(*device_ids)
            rc = lib.axon_start_nrt_profile(ids, len(device_ids))
        else:
            rc = lib.axon_start_nrt_profile(None, 0)
        if rc != 0:
            raise RuntimeError(f"axon_start_nrt_profile rc={rc}")
        try:
            yield
        finally:
            n = lib.axon_stop_nrt_profile(str(output_dir).encode())
            print(f"profile: {n} file(s) in {output_dir}", file=sys.stderr)

    mod = types.ModuleType("antenv.axon_hooks")
    holder = {"h": _hook}
    mod.set_axon_ntff_profile_hook = lambda h: holder.__setitem__("h", h)
    mod.get_axon_ntff_profile_hook = lambda: holder.get("h")
    sys.modules["antenv.axon_hooks"] = mod
    import antenv
    antenv.axon_hooks = mod
    import concourse.bass_utils as bu
    bu.upload_artifacts = lambda tmpdir: str(tmpdir)
    return True



# ---------------------------------------------------------------- graph
def build_nc():
    nc = bacc.Bacc("TRN2", target_bir_lowering=False, debug=False,
                   num_devices=N_CORES)

    rotm = nc.dram_tensor("rotm", [D, D], BF, kind="ExternalInput").ap()
    xt = nc.dram_tensor("xt", [DM, S], BF, kind="ExternalInput").ap()
    wq = nc.dram_tensor("wq", [DM, HD_CORE], BF, kind="ExternalInput").ap()
    wk = nc.dram_tensor("wk", [DM, D], BF, kind="ExternalInput").ap()
    wv = nc.dram_tensor("wv", [DM, D], BF, kind="ExternalInput").ap()
    wo = nc.dram_tensor("wo", [DM, HD_CORE], BF, kind="ExternalInput").ap()
    cosq = nc.dram_tensor("cosq", [D, S], F32, kind="ExternalInput").ap()
    sinq = nc.dram_tensor("sinq", [D, S], F32, kind="ExternalInput").ap()
    cosk = nc.dram_tensor("cosk", [D, S], F32, kind="ExternalInput").ap()
    sink = nc.dram_tensor("sink", [D, S], F32, kind="ExternalInput").ap()
    out = nc.dram_tensor("out", [S, HD_CORE], F32, kind="ExternalOutput").ap()

    groups = [[0, 1, 2, 3], [4, 5, 6, 7]]

    with tile.TileContext(nc) as tc:
        with tc.tile_pool(name="const", bufs=1) as cpool, \
             tc.tile_pool(name="tblp", bufs=2) as tpool, \
             tc.tile_pool(name="wts", bufs=1) as wpool, \
             tc.tile_pool(name="acts", bufs=1) as apool, \
             tc.tile_pool(name="xin", bufs=32) as xpool, \
             tc.tile_pool(name="work", bufs=3) as work, \
                          tc.tile_pool(name="etwork", bufs=9) as etwork, \
             tc.tile_pool(name="ogp", bufs=32) as ogpool, \
             tc.tile_pool(name="stats", bufs=2) as stats, \
             tc.tile_pool(name="bcp", bufs=2) as bcpool, \
             tc.tile_pool(name="psmm", bufs=4, space="PSUM") as ps_mm, \
             tc.tile_pool(name="pspv", bufs=2, space="PSUM") as ps_pv, \
             tc.tile_pool(name="psden", bufs=1, space="PSUM") as ps_den, \
             tc.tile_pool(name="pstr", bufs=1, space="PSUM") as ps_tr, \
             tc.tile_pool(name="dram", bufs=1, space="DRAM") as dpool:

            # warm up the collective path FIRST: the gpsimd engine is the
            # collective-trigger path, so nothing may sit ahead of this in
            # its queue (a late trigger delays the group-wide init barrier
            # and cascades into every later gather)
            warm_in = dpool.tile([128, 4], F32, tag="warm_in", name="warm_in")
            warm_out = dpool.tile([4, 128, 4], F32, tag="warm_out",
                                  name="warm_out")
            nc.gpsimd.dma_start(out=warm_in[:], in_=cosq[0:128, 0:4])
            nc.gpsimd.collective_compute(
                "AllGather", mybir.AluOpType.bypass,
                replica_groups=groups,
                ins=[warm_in.opt()], outs=[warm_out.opt()])

            # HAM warm-up: ~100 dummy matmuls on a zero tile keep the PE
            # busy through the initial DMA ramp so the clock gate opens
            # (4/8 -> 8/8) before the first real projection matmuls
            hamw = cpool.tile([128, 128], BF, tag="hamw", name="hamw")
            nc.vector.memset(hamw[:], 0.0)
            for _ in range(60):
                psd = ps_mm.tile([128, 512], F32, tag="mm", name="mm")
                nc.tensor.matmul(psd[:, 0:128], hamw[:], hamw[:],
                                 start=True, stop=True)

            # ---------------- constants
            ident = cpool.tile([128, 128], BF, tag="ident", name="ident")
            make_identity(nc, ident[:])
            # transposed causal mask: keep [k_row p, q_col j] iff j >= p
            cmaskT = cpool.tile([128, 128], F32, tag="cmaskT", name="cmaskT")
            nc.gpsimd.memset(cmaskT[:], 0.0)
            nc.gpsimd.affine_select(
                out=cmaskT[:], in_=cmaskT[:],
                compare_op=mybir.AluOpType.is_ge, fill=NEG,
                base=0, pattern=[[1, 128]], channel_multiplier=-1)
            ones_sb = cpool.tile([128, 1], BF, tag="ones", name="ones")
            nc.gpsimd.memset(ones_sb[:], 1.0)
            # warm the ACT exp table during the DMA ramp (the first real exp
            # otherwise pays the ~2.7us table load inside attn(0)'s chain)
            warm_act = cpool.tile([1, 4], F32, tag="warm_act", name="warm_act")
            nc.scalar.activation(out=warm_act[0:1, 0:1], in_=ones_sb[0:1, 0:1],
                                 func=mybir.ActivationFunctionType.Exp)
            # signed rotate-half permutation (lhsT): sh = rotm.T @ raw
            rot_sb = cpool.tile([D, D], BF, tag="rotm", name="rotm")

            # ---------------- weights + first x chunk (load order = use order)
            wq_sb = [wpool.tile([128, HD_CORE], BF, tag=f"wq{kt}",
                                name=f"wq{kt}") for kt in range(KT)]
            wk_sb = [wpool.tile([128, D], BF, tag=f"wk{kt}",
                                name=f"wk{kt}") for kt in range(KT)]
            wv_sb = [wpool.tile([128, D], BF, tag=f"wv{kt}",
                                name=f"wv{kt}") for kt in range(KT)]
            wo_sb = [wpool.tile([128, HD_CORE], BF, tag=f"wo{kt}",
                                name=f"wo{kt}") for kt in range(KT)]

            def load_xc(c):
                ts_ = [xpool.tile([128, 512], BF, tag="xc", name="xc")
                       for _ in range(KT)]
                for kt in range(KT):
                    # interleave weight loads on first chunk so matmul kt can
                    # start as soon as its slices land
                    nc.sync.dma_start(
                        out=ts_[kt][:],
                        in_=xt[ds(128 * kt, 128), ds(512 * c, 512)])
                    if c == 0:
                        nc.sync.dma_start(out=wk_sb[kt][:],
                                          in_=wk[ds(128 * kt, 128), :])
                return ts_

            def load_tbl(c):
                """rope-table slices for chunk c (sync queue, use order)."""
                tb = {}
                for name, src in (("cosk", cosk), ("sink", sink),
                                  ("cosq", cosq), ("sinq", sinq)):
                    t = tpool.tile([D, 512], F32, tag=f"tb_{name}")
                    nc.sync.dma_start(out=t[:], in_=src[:, ds(512 * c, 512)])
                    tb[name] = t
                return tb

            xc_state = [load_xc(0)]
            # rot matrix first (the rope perm-matmul sits in the PE FIFO
            # right after K-proj), then wq so Q-proj head 0 can start as
            # soon as K-proj drains; tables and wv follow
            nc.sync.dma_start(out=rot_sb[:], in_=rotm[:])
            for kt in range(KT):
                nc.sync.dma_start(out=wq_sb[kt][:],
                                  in_=wq[ds(128 * kt, 128), :])
            tbl_state = [load_tbl(0)]
            for kt in range(KT):
                nc.sync.dma_start(out=wv_sb[kt][:],
                                  in_=wv[ds(128 * kt, 128), :])

            # ---------------- persistent activations
            qt_sb = [apool.tile([D, S], BF, tag=f"qt{h}", name=f"qt{h}")
                     for h in range(G)]
            kt_sb = apool.tile([D, S], BF, tag="kt", name="kt")
            vtok_sb = apool.tile([128, TOKB, D], BF, tag="vtok", name="vtok")

            # ---------------- projections + RoPE + v transpose
            def rope_store(raw_ps, dst_slice, cos_t, sin_t):
                # rotate-half via a PE permutation matmul (no DMA in chain);
                # t2/add ride the idle gpsimd ALU to decongest VectorE
                raw = work.tile([128, 512], BF, tag="qraw", name="qraw")
                nc.scalar.copy(raw[:], raw_ps[:])
                sh_ps = ps_pv.tile([128, 512], F32, tag="pv", name="ropesh")
                nc.tensor.matmul(sh_ps[:], rot_sb[:], raw[:],
                                 start=True, stop=True)
                t1 = work.tile([128, 512], F32, tag="t1", name="t1")
                nc.vector.tensor_mul(t1[:], sh_ps[:], sin_t[:])
                t2 = work.tile([128, 512], F32, tag="t2", name="t2")
                nc.vector.tensor_mul(t2[:], raw[:], cos_t[:])
                nc.vector.tensor_add(dst_slice, t1[:], t2[:])

            def emit_proj(c):
                xc = xc_state.pop(0)
                tbl = tbl_state.pop(0)
                # k
                ps = ps_mm.tile([128, 512], F32, tag="mm", name="mm")
                for kt in range(KT):
                    nc.tensor.matmul(ps[:], wk_sb[kt][:], xc[kt][:],
                                     start=(kt == 0), stop=(kt == KT - 1))
                rope_store(ps, kt_sb[:, ds(512 * c, 512)],
                           tbl["cosk"], tbl["sink"])
                # q heads
                for h in range(G):
                    ps = ps_mm.tile([128, 512], F32, tag="mm", name="mm")
                    for kt in range(KT):
                        nc.tensor.matmul(ps[:], wq_sb[kt][:, ts(h, 128)],
                                         xc[kt][:],
                                         start=(kt == 0), stop=(kt == KT - 1))
                    rope_store(ps, qt_sb[h][:, ds(512 * c, 512)],
                               tbl["cosq"], tbl["sinq"])
                # v last (no rope; transpose to token-major)
                ps = ps_mm.tile([128, 512], F32, tag="mm", name="mm")
                for kt in range(KT):
                    nc.tensor.matmul(ps[:], wv_sb[kt][:], xc[kt][:],
                                     start=(kt == 0), stop=(kt == KT - 1))
                vst = work.tile([128, 512], BF, tag="vst", name="vst")
                nc.scalar.copy(vst[:], ps[:])
                trp = ps_tr.tile([128, 512], BF, tag="tr", name="trv")
                for j in range(4):
                    nc.tensor.transpose(trp[:, ts(j, 128)], vst[:, ts(j, 128)],
                                        ident[:])
                nc.vector.tensor_copy(out=vtok_sb[:, ds(4 * c, 4), :], in_=trp[:])
                if c + 1 < TCH:
                    xc_state.append(load_xc(c + 1))
                    tbl_state.append(load_tbl(c + 1))

            # ---------------- attention, scores computed pre-transposed
            # quarters 0-2 gather all 4 local heads at quarter end; quarter 3
            # gathers per head so the final Wo only waits on 128KB.
            cin_q = [dpool.tile([D, G, 512], BF, tag=f"cinq{t}", name=f"cinq{t}")
                     for t in range(3)]
            cout_q = [dpool.tile([4, D, G, 512], BF, tag=f"coutq{t}",
                                 name=f"coutq{t}") for t in range(3)]
            cin_q3 = [dpool.tile([D, 2, 512], BF, tag=f"cinq3p{p}",
                                 name=f"cinq3p{p}") for p in range(2)]
            cout_q3 = [dpool.tile([4, D, 2, 512], BF, tag=f"coutq3p{p}",
                                  name=f"coutq3p{p}") for p in range(2)]

            def wo_quarter(t, part=None):
                """Load og tiles for quarter t (part: q3 head-pair 0/1)."""
                if t < 3:
                    srcs = [(r * G + h, cout_q[t][r, :, h, :])
                            for r in range(4) for h in range(G)]
                else:
                    srcs = [(r * G + 2 * part + h, cout_q3[part][r, :, h, :])
                            for h in range(2) for r in range(4)]
                srcs.sort()
                ogs = []
                for kt, ap in srcs:
                    og = ogpool.tile([128, 512], BF, tag="og", name="og")
                    # gpsimd queue: a pending gather-wait here must not block
                    # the sync queue's x/weight streams (head-of-line)
                    nc.gpsimd.dma_start(out=og[:], in_=ap)
                    ogs.append((kt, og))
                return ogs

            def wo_mm(t, ogs, ogs2=None):
                """ogs2: late-arriving og tiles; all chains' ogs-matmuls are
                emitted before any ogs2 matmul so the PE stalls at most once
                on the final gather."""
                n1 = len(ogs)
                pws = []
                for tb in range(4):
                    pw = ps_mm.tile([128, 512], F32, tag="mm", name="mm")
                    pws.append(pw)
                    for idx, (kt, og) in enumerate(ogs):
                        nc.tensor.matmul(pw[:], og[:, ts(tb, 128)],
                                         wo_sb[kt][:],
                                         start=(idx == 0), stop=(idx == 15))
                for tb in range(4):
                    pw = pws[tb]
                    if ogs2 is not None:
                        for j, (kt, og) in enumerate(ogs2):
                            nc.tensor.matmul(pw[:], og[:, ts(tb, 128)],
                                             wo_sb[kt][:],
                                             start=False,
                                             stop=(n1 + j == 15))
                    ost = work.tile([128, 512], F32, tag="ost", name="ost")
                    nc.scalar.copy(ost[:], pw[:])
                    nc.sync.dma_start(
                        out=out[ds(512 * t + 128 * tb, 128), :], in_=ost[:])

            def emit_st(h, qc, kb):
                """score block, transposed: [k 128, q<=512] -> exp -> et"""
                band = kb - 4 * qc
                et = etwork.tile([128, 512], BF, tag="et", name="et")
                sps = ps_mm.tile([128, 512], F32, tag="mm", name="mm")
                if band >= 0:
                    off = 128 * band
                    w = 512 - off
                    nc.tensor.matmul(sps[:, :w], kt_sb[:, ts(kb, 128)],
                                     qt_sb[h][:, ds(512 * qc + off, w)],
                                     start=True, stop=True)
                    nc.vector.tensor_add(sps[:, :128], sps[:, :128], cmaskT[:])
                    if off:
                        nc.vector.memset(et[:, :off], 0.0)
                    nc.scalar.activation(
                        out=et[:, ds(off, w)], in_=sps[:, :w],
                        func=mybir.ActivationFunctionType.Exp)
                    return et, off
                nc.tensor.matmul(sps[:], kt_sb[:, ts(kb, 128)],
                                 qt_sb[h][:, ds(512 * qc, 512)],
                                 start=True, stop=True)
                nc.scalar.activation(
                    out=et[:], in_=sps[:],
                    func=mybir.ActivationFunctionType.Exp)
                return et, 0

            wo_pend = {}
            # wo og-loads / matmuls for quarter t ride inside later quarters'
            # head loops, leaving ~100us of slack after each gather's trigger
            # so collective-start skew never stalls the PE; quarters 1-3 run
            # after attn(3) as ~45us of PE fill overlapping the final gathers
            loads_at = {(2, 3): 0, (3, 0): 1}
            mms_at = {}

            def emit_attn(qc):
                for h in range(G):
                    if (qc, h) in loads_at:
                        t = loads_at[(qc, h)]
                        wo_pend[t] = wo_quarter(t)
                    if (qc, h) in mms_at:
                        t = mms_at[(qc, h)]
                        wo_mm(t, wo_pend.pop(t))
                    nkb = 4 * qc + 4
                    oT_ps = ps_pv.tile([128, 512], F32, tag="pv", name="pv")
                    den_ps = ps_den.tile([1, 512], F32, tag="den", name="den")
                    pend = [emit_st(h, qc, k) for k in range(min(4, nkb))]
                    ngrp = (nkb + 3) // 4
                    esum = None
                    gready = []
                    emitted = 0
                    for kb in range(nkb):
                        et, off = pend.pop(0)
                        if kb + 4 < nkb:
                            pend.append(emit_st(h, qc, kb + 4))
                        nc.tensor.matmul(oT_ps[:, ds(off, 512 - off)],
                                         vtok_sb[:, kb, :],
                                         et[:, ds(off, 512 - off)],
                                         start=(kb == 0), stop=(kb == nkb - 1))
                        # denominator: sum groups of 4 et tiles on DVE; each
                        # group's ones-matmul is emitted one group LATE so
                        # the PE FIFO never waits on the DVE chain
                        gi, gj = divmod(kb, 4)
                        last_in_grp = (gj == 3 or kb == nkb - 1)
                        if gj == 0:
                            esum = et
                        else:
                            nsum = etwork.tile([128, 512], BF, tag="esum",
                                               name="esum", bufs=9)
                            nc.vector.tensor_add(nsum[:], esum[:], et[:])
                            esum = nsum
                        if last_in_grp:
                            gready.append(esum)
                            if len(gready) > 1:
                                nc.tensor.matmul(den_ps[:], ones_sb[:, 0:1],
                                                 gready.pop(0)[:],
                                                 start=(emitted == 0),
                                                 stop=False)
                                emitted += 1
                    for es in gready:
                        nc.tensor.matmul(den_ps[:], ones_sb[:, 0:1], es[:],
                                         start=(emitted == 0),
                                         stop=(emitted == ngrp - 1))
                        emitted += 1
                    den_sb = stats.tile([1, 512], F32, tag="den_sb",
                                        name="den_sb")
                    nc.scalar.copy(den_sb[:], den_ps[:])
                    rec = stats.tile([1, 512], F32, tag="recq", name="recq")
                    nc.vector.reciprocal(rec[:], den_sb[:])
                    bcast = bcpool.tile([128, 512], F32, tag="bcast",
                                        name="bcast")
                    nc.gpsimd.partition_broadcast(bcast[:], rec[:])
                    otst = work.tile([128, 512], BF, tag="otst", name="otst")
                    nc.vector.tensor_mul(otst[:], oT_ps[:], bcast[:])
                    if qc < 3:
                        nc.sync.dma_start(out=cin_q[qc][:, h, :], in_=otst[:])
                    else:
                        p, hh = divmod(h, 2)
                        nc.sync.dma_start(out=cin_q3[p][:, hh, :], in_=otst[:])
                        if hh == 1:
                            nc.gpsimd.collective_compute(
                                "AllGather", mybir.AluOpType.bypass,
                                replica_groups=groups,
                                ins=[cin_q3[p].opt()],
                                outs=[cout_q3[p].opt()])
                if qc < 3:
                    nc.gpsimd.collective_compute(
                        "AllGather", mybir.AluOpType.bypass,
                        replica_groups=groups,
                        ins=[cin_q[qc].opt()], outs=[cout_q[qc].opt()])

            emit_proj(0)
            emit_attn(0)
            emit_proj(1)
            # wo weights: needed from the first Wo quarter (inside attn(1))
            for kt in range(KT):
                nc.sync.dma_start(out=wo_sb[kt][:],
                                  in_=wo[ds(128 * kt, 128), :])
            emit_attn(1)
            emit_proj(2)
            emit_attn(2)
            emit_proj(3)
            emit_attn(3)
            # tail: ALL wo work runs after attn(3) so the final head-pair
            # gathers trigger as early as possible and complete under ~60us
            # of Wo matmul fill.  og-load order keeps the gpsimd DMA queue
            # from blocking ahead of the pair triggers.
            wo_mm(0, wo_pend.pop(0))
            wo_mm(1, wo_pend.pop(1))
            wo_mm(2, wo_quarter(2))
            wo_mm(3, wo_quarter(3, part=0), wo_quarter(3, part=1))

    nc.finalize()
    return nc


_NC_CACHE = {}


def _get_nc():
    if "nc" not in _NC_CACHE:
        _NC_CACHE["nc"] = build_nc()
    return _NC_CACHE["nc"]


def _rope_tables():
    inv = 1.0 / (THETA ** (np.arange(0, D, 2, dtype=np.float64) / D))  # [64]
    pos = np.arange(S, dtype=np.float64)
    fr = pos[:, None] * inv[None, :]                 # [S, 64]
    emb = np.concatenate([fr, fr], axis=1)           # [S, D]
    cos = np.cos(emb).T.astype(np.float32)           # [D, S]
    sin = np.sin(emb).T.astype(np.float32)           # sign lives in rotm
    scale = np.float32(D ** -0.5)
    return (cos * scale, sin * scale,                # q tables (pre-scaled)
            cos.copy(), sin.copy())                  # k tables


def _rot_matrix():
    """lhsT for sh = rotate_half(raw): sh[i] = -raw[i+64] (i<64),
    raw[i-64] (i>=64);  rotm[p, i] so that sh = rotm.T @ raw."""
    m = np.zeros((D, D), dtype=np.float32)
    half = D // 2
    for i in range(half):
        m[i + half, i] = -1.0
        m[i, i + half] = 1.0
    return m.astype(BF16)


def kernel(x, Wq, Wk, Wv, Wo):
    global LAST_EXEC_TIME_NS, LAST_RESULTS
    nc = _get_nc()
    cq, sq, ck, sk = _rope_tables()
    rm = _rot_matrix()
    in_maps = []
    for c in range(N_CORES):
        b, kh = c // 4, c % 4
        in_maps.append({
            "rotm": rm,
            "xt": np.ascontiguousarray(x[b].T).astype(BF16),
            "wq": np.ascontiguousarray(Wq[:, kh * HD_CORE:(kh + 1) * HD_CORE]).astype(BF16),
            "wk": np.ascontiguousarray(Wk[:, kh * D:(kh + 1) * D]).astype(BF16),
            "wv": np.ascontiguousarray(Wv[:, kh * D:(kh + 1) * D]).astype(BF16),
            "wo": np.ascontiguousarray(Wo[:, kh * HD_CORE:(kh + 1) * HD_CORE]).astype(BF16),
            "cosq": cq, "sinq": sq, "cosk": ck, "sink": sk,
        })
    trace = os.environ.get("KERNEL_TRACE", "0") == "1" and _install_ntff_hook()
    res = run_bass_kernel_spmd(nc, in_maps, core_ids=list(range(N_CORES)),
                               trace=trace)
    LAST_EXEC_TIME_NS = res.exec_time_ns
    LAST_RESULTS = res
    out = np.empty((B, S, DM), dtype=np.float32)
    for c in range(N_CORES):
        b, kh = c // 4, c % 4
        out[b, :, kh * HD_CORE:(kh + 1) * HD_CORE] = res.results[c]["out"]
    return out
